# revision 1
# baseline (speedup 1.0000x reference)
"""GQA attention (RoPE + causal softmax + o_proj) on 8 Trainium2 NeuronCores.

Sharding: core = b*4 + g where b = batch (2), g = head-group (4).
Each core handles 8 query heads (global 8g..8g+7) and their 2 KV heads
(2g, 2g+1) for one batch element, producing a partial o_proj output
(contraction over its 512 of the 2048 hd dims). The host sums the 4
partials per batch element (o_part is bf16; host upcasts).

Per-core layout/schedule (all matmul operands bf16, fp32 PSUM accum):
  - Inputs are host-packed into 4 flat [128, N] tensors so the whole
    input load is 5 DMA instructions, issued from sync/scalar/vector
    queues in parallel (DMA issue costs ~0.7us of the issuing engine's
    queue REGARDLESS of size, so many small input DMAs serialize the
    sync queue and block the rope-swap DMAs behind them).
  - q^T/k^T built per 128-row chunk pairing heads (i, i+4); scores are
    computed transposed (S^T[k,q]) as two row-tiled K=64 matmuls that
    run concurrently in the PE array.
  - AV stationary vnat[kb] = [v0(0:64) | 1 | 0*63 | v1(128:192)]:
      av0 = vnat[:,0:65].T  @ pt0 -> v0 at partitions 0:64, den0 at 64
      av1 = vnat[:,64:192].T @ pt1 -> den1 at partition 0, v1 at 64:128
    so attnT rows 64:128 are written lane-aligned (no SBUF->SBUF DMA)
    and den1 feeds reciprocal/broadcast without a partition move.
  - At pg end avs are evicted to SBUF immediately (frees the 2 "av"
    PSUM banks for the next pg), then den->recip->broadcast->mul chains
    run off SBUF (muls on GPSIMD; recip/evict on DVE).
  - Schedule: proj(0) dense, then attention chunk c interleaves
    proj(c+1) + o_proj(c-1) units as PE filler (own "fil" PSUM slots)
    so ScalarE exp (the per-kb rate limiter) starts early and PE never
    drains. PSUM: st 2x2 + av 2 + fil 2 = 8 banks.
  - Engine balance: exp exclusively on ACT; PSUM reads (qraw/t1/
    evictions) on DVE; rope t2+add, diag-mask half, norm muls on GPSIMD.
"""

import numpy as np
import ml_dtypes
from contextlib import ExitStack

import concourse.mybir as mybir
from concourse import bacc
from concourse.tile import TileContext
from concourse.bass_utils import run_bass_kernel_spmd

BF16 = mybir.dt.bfloat16
F32 = mybir.dt.float32
NP_BF16 = ml_dtypes.bfloat16

HID = 2048
D = 64
H = 32           # global query heads
KV = 8           # global kv heads
B = 2
P = 128
SC = 512         # q-chunk width (also matmul free dim / PSUM bank)

_CACHE = {}


def build_nc(S):
    assert S % SC == 0
    NHID = HID // P       # hid chunks (16)
    NSB = S // P          # 128-row s-blocks
    NSC = S // SC         # 512-col s-chunks
    QCH = 4               # q chunk-pairs
    EXP = mybir.ActivationFunctionType.Exp

    nc = bacc.Bacc("TRN2", target_bir_lowering=False, debug=False)
    # host-packed flat inputs (see prep_core_inputs for layouts)
    xTp = nc.dram_tensor("xTp", [P, NHID * S], BF16, kind="ExternalInput")
    wqkvp = nc.dram_tensor("wqkvp", [P, NHID * 768], BF16, kind="ExternalInput")
    csm = nc.dram_tensor("csm", [P, 2 * S + 3 * P], BF16, kind="ExternalInput")
    wop = nc.dram_tensor("wop", [P, 4 * HID], BF16, kind="ExternalInput")
    o_part = nc.dram_tensor("o_part", [S, HID], BF16, kind="ExternalOutput")

    with TileContext(nc) as tc, ExitStack() as ctx:
        res = ctx.enter_context(tc.tile_pool(name="res", bufs=1))
        rope = ctx.enter_context(tc.tile_pool(name="rope", bufs=3))
        ptp = ctx.enter_context(tc.tile_pool(name="ptp", bufs=5))
        nrm = ctx.enter_context(tc.tile_pool(name="nrm", bufs=2))
        obp = ctx.enter_context(tc.tile_pool(name="obp", bufs=2))
        psum = ctx.enter_context(tc.tile_pool(name="psum", bufs=1, space="PSUM"))

        # ---- input DMA: purpose-split queues. sync: xT chunk 0
        # (h-progressive quarters); scalar: weights m-major-progressive;
        # gpsimd: cos/sin/mask (small, needed by the first rope). The
        # rope-swap / den / o_part DMAs later share these queues by
        # latency class.
        xtall = res.tile([P, NHID * S], BF16, tag="xtall", name="xtall")
        wqall = res.tile([P, NHID * 768], BF16, tag="wqall", name="wqall")
        csm_t = res.tile([P, 2 * S + 3 * P], BF16, tag="csm", name="csm")
        for q in range(4):      # xT c0: h-quarter q arrives early
            xsl = slice(q * 4 * SC, (q + 1) * 4 * SC)
            nc.sync.dma_start(out=xtall[:, xsl], in_=xTp[:, xsl])
            if q == 0:          # cos/sin/mask/perm ride the idle sync queue
                nc.sync.dma_start(out=csm_t, in_=csm[:, :])
        for b in range(6):      # weight blocks in consumption order
            wsl = slice(b * NHID * P, (b + 1) * NHID * P)
            nc.scalar.dma_start(out=wqall[:, wsl], in_=wqkvp[:, wsl])
        nc.scalar.dma_start(
            out=xtall[:, NHID * SC:], in_=xTp[:, NHID * SC:]
        )
        woall = res.tile([P, 4 * HID], BF16, tag="woall", name="woall")
        nc.scalar.dma_start(out=woall, in_=wop[:, :])

        # packed-layout views
        def xtv(h, s):      # x^T [hid chunk h, s-chunk s] -> [128, 512]
            base = (s * NHID + h) * SC
            return xtall[:, base:base + SC]

        def xtvb(h, sb):    # x^T [hid chunk h, s-block sb] -> [128, 128]
            base = ((sb // 4) * NHID + h) * SC + (sb % 4) * P
            return xtall[:, base:base + P]

        # wqall column layout: 6 blocks x 16 h x 128 cols, block order
        # (k, q-pair0, v, q-pair1, q-pair2, q-pair3) = consumption order
        _WBLK = {4: 0, 0: 1, 1: 3, 2: 4, 3: 5}

        def wqv(h, m):      # wqkv [hid chunk h, col chunk m] (m=4 -> k)
            base = (_WBLK[m] * NHID + h) * P
            return wqall[:, base:base + P]

        def wvv(h):         # wv [hid chunk h] -> [128, 128]
            base = (2 * NHID + h) * P
            return wqall[:, base:base + P]

        cos_sb = csm_t[:, 0:S]
        sin_sb = csm_t[:, S:2 * S]
        mask_sb = csm_t[:, 2 * S:2 * S + P]
        perm_sb = csm_t[:, 2 * S + P:2 * S + 2 * P]   # rotate-half swap
        ones_sb = csm_t[:, 2 * S + 2 * P:2 * S + 3 * P]

        def wov(i, n):      # wo [hd chunk i, hid cols n*SC..] -> [128, 512]
            return woall[:, i * HID + n * SC: i * HID + (n + 1) * SC]

        # chunks 0-3: q head pairs (i, i+4); chunk 4: k (kv0 rows 0-63, kv1 64-127)
        qkrot = []
        for m in range(5):
            t = res.tile([P, S], BF16, tag=f"qkrot{m}", name=f"qkrot{m}")
            qkrot.append(t)
        # v tiles [128, 192]: [v0(0:64) | 1 | 0*63 | v1(128:192)]
        vnat = [res.tile([P, 192], BF16, tag=f"vnat{sb}", name=f"vnat{sb}")
                for sb in range(NSB)]
        attnT = []
        for i in range(QCH):
            t = res.tile([P, S], BF16, tag=f"attnT{i}", name=f"attnT{i}")
            attnT.append(t)

        def gen_o_chunk(c, ptag="fil"):
            for qb in range(4 * c, 4 * c + 4):
                ob = obp.tile([P, HID], BF16, tag="ob", name="ob")
                for n in range(HID // SC):
                    po = psum.tile([P, SC], F32, tag=ptag, bufs=2, name="po")
                    for i in range(QCH):
                        nc.tensor.matmul(
                            po,
                            lhsT=attnT[i][:, qb * P:(qb + 1) * P],
                            rhs=wov(i, n),
                            start=(i == 0),
                            stop=(i == QCH - 1),
                        )
                    nc.vector.tensor_copy(ob[:, n * SC:(n + 1) * SC], po)
                    yield
                nc.scalar.dma_start(out=o_part[qb * P:(qb + 1) * P, :], in_=ob)

        def gen_proj_munits(s, ms):
            """Emit projection m-chunk units for s-chunk s (yields).

            Matmul groups stay consecutive; eviction units only follow
            completed groups. Rope eviction split: qraw/t1 on DVE (the
            PSUM readers), t2 + final add on GPSIMD so the DVE FIFO
            never head-of-line-blocks on the qswp DMA chain."""
            sl = slice(s * SC, (s + 1) * SC)
            for m in ms:
                ps = psum.tile([P, SC], F32, tag="fil", bufs=2, name="ps_proj")
                for h0 in (0, 8):
                    for h in range(h0, h0 + 8):
                        nc.tensor.matmul(
                            ps,
                            lhsT=wqv(h, m),
                            rhs=xtv(h, s),
                            start=(h == 0),
                            stop=(h == NHID - 1),
                        )
                    yield
                # rotate_half: engines are lane-locked, so the +-32-
                # partition swap runs on the PE as a constant permutation
                # matmul (no DMA; sign lives in sinT host-side)
                qraw = rope.tile([P, SC], BF16, tag="qraw", bufs=3, name="qraw")
                nc.vector.tensor_copy(qraw, ps)
                t1 = rope.tile([P, SC], BF16, tag="t1", bufs=3, name="t1")
                nc.vector.tensor_mul(t1, ps, cos_sb[:, sl])
                ps2 = psum.tile([P, SC], F32, tag="fil", bufs=2, name="ps_rot")
                nc.tensor.matmul(ps2, lhsT=perm_sb, rhs=qraw,
                                 start=True, stop=True)
                t2 = rope.tile([P, SC], BF16, tag="t2", bufs=3, name="t2")
                nc.vector.tensor_mul(t2, ps2, sin_sb[:, sl])
                nc.gpsimd.tensor_add(qkrot[m][:, sl], t1, t2)
                yield

        def gen_v_units(s):
            for sb in range(4 * s, 4 * s + 4):
                t = vnat[sb]
                nc.gpsimd.memset(t[:, 64:65], 1.0)
                nc.gpsimd.memset(t[:, 65:128], 0.0)
                pv = psum.tile([P, 128], F32, tag="fil", bufs=2, name="ps_v")
                for h in range(NHID):
                    nc.tensor.matmul(
                        pv,
                        lhsT=xtvb(h, sb),
                        rhs=wvv(h),
                        start=(h == 0),
                        stop=(h == NHID - 1),
                    )
                yield
                nc.vector.tensor_copy(t[:, 0:64], pv[:, 0:64])
                nc.vector.tensor_copy(t[:, 128:192], pv[:, 64:128])
                yield

        def gen_proj_head(s):   # pg0 deps of chunk s: k, q-pair 0, v
            yield from gen_proj_munits(s, (4, 0))
            yield from gen_v_units(s)

        def gen_proj_tail(s):   # pg1-3 deps: fillered into chunk s itself
            yield from gen_proj_munits(s, (1, 2, 3))

        def interleave(*gens):
            gens = [g for g in gens if g is not None]
            i = 0
            while gens:
                g = gens[i % len(gens)]
                try:
                    next(g)
                except StopIteration:
                    gens.remove(g)
                    continue
                yield
                i += 1

        def winterleave(specs):
            """Round-robin with weights: (gen, weight) pulls weight
            units per round. Tail-proj gets weight 2 so qkrot[m] for
            upcoming head-pairs lands ahead of their first scores."""
            active = [(g, w) for g, w in specs if g is not None]
            while active:
                nxt = []
                for g, w in active:
                    alive = True
                    for _ in range(w):
                        try:
                            next(g)
                        except StopIteration:
                            alive = False
                            break
                        yield
                    if alive:
                        nxt.append((g, w))
                active = nxt

        # ---- proj head of chunk 0 dense (nothing to overlap with) ----
        with nc.named_scope("projA0"):
            for _ in gen_proj_head(0):
                pass

        # deferred o-unit generators: chunk c may drain earlier chunks'
        # o units up to a per-chunk cap; the rest spill to later chunks
        o_gens = []
        o_caps = {0: 0, 1: 16, 2: 16, 3: 16}

        def gen_o_budget(cap):
            spent = 0
            while o_gens and spent < cap:
                try:
                    next(o_gens[0])
                except StopIteration:
                    o_gens.pop(0)
                    continue
                spent += 1
                yield

        # ---- attention chunks; proj tail(c) + head(c+1) + deferred o
        # drip into the kb loop as PE fill work while ScalarE streams
        # exps ----
        for c in range(NSC):
          with nc.named_scope(f"attn_c{c}"):
            q0 = c * SC
            nkb = 4 * c + 4
            if c >= 1:
                o_gens.append(gen_o_chunk(c - 1))
            filler = winterleave([
                (gen_proj_tail(c), 2),
                (gen_proj_head(c + 1) if c + 1 < NSC else None, 1),
                (gen_o_budget(o_caps[c]), 1),
            ])
            n_units = 9 + (10 if c + 1 < NSC else 0) + (16 if c >= 1 else 0)
            total_iters = 4 * nkb
            it = 0
            spent = 0
            for pg in (0, 1, 2, 3):
                av0 = psum.tile([P, SC], F32, tag="av", bufs=2, name="av0")
                av1 = psum.tile([P, SC], F32, tag="av", bufs=2, name="av1")
                hp = pg

                def emit_av(kb, pt, vs):
                    nc.tensor.matmul(
                        av0[0:65, vs:SC],
                        lhsT=vnat[kb][:, 0:65],
                        rhs=pt[:, vs:SC],
                        start=(kb == first_kb), stop=(kb == last_kb),
                    )
                    nc.tensor.matmul(
                        av1[:, vs:SC],
                        lhsT=vnat[kb][:, 64:192],
                        rhs=pt[:, SC + vs:2 * SC],
                        start=(kb == first_kb), stop=(kb == last_kb),
                    )

                # software pipeline: AV(kb-3) is emitted after scores(kb),
                # giving each exp ~two iterations of cover
                pending = []
                kb_order = list(range(4 * c, nkb)) + list(range(0, 4 * c))
                first_kb, last_kb = kb_order[0], kb_order[-1]
                for kb in kb_order:
                    vs = max(0, (kb - 4 * c) * P)  # first valid col in chunk
                    st = psum.tile([P, 2 * SC], F32, tag="st", bufs=2, name="st")
                    nc.tensor.matmul(
                        st[:, vs:SC],
                        lhsT=qkrot[4][0:64, kb * P:(kb + 1) * P],
                        rhs=qkrot[hp][0:64, q0 + vs:q0 + SC],
                        start=True, stop=True,
                    )
                    nc.tensor.matmul(
                        st[:, SC + vs:2 * SC],
                        lhsT=qkrot[4][64:128, kb * P:(kb + 1) * P],
                        rhs=qkrot[hp][64:128, q0 + vs:q0 + SC],
                        start=True, stop=True,
                    )
                    if len(pending) >= 3:
                        emit_av(*pending.pop(0))
                    pt = ptp.tile([P, 2 * SC], BF16, tag="pt", name="pt")
                    # one exp over [vs:1024]: the dead span [SC:SC+vs] is
                    # unwritten PSUM (may exp to junk; never read)
                    nc.scalar.activation(
                        pt[:, vs:2 * SC], st[:, vs:2 * SC], EXP, scale=0.125
                    )
                    if kb - 4 * c >= 0:  # diagonal block: mask triangle
                        nc.vector.tensor_mul(
                            pt[:, vs:vs + P], pt[:, vs:vs + P], mask_sb
                        )
                        nc.gpsimd.tensor_mul(
                            pt[:, SC + vs:SC + vs + P],
                            pt[:, SC + vs:SC + vs + P], mask_sb
                        )
                    pending.append((kb, pt, vs))
                    it += 1
                    want = (it * n_units) // total_iters
                    while spent < want:
                        try:
                            next(filler)
                            spent += 1
                        except StopIteration:
                            spent = want
                            break
                for pp in pending:
                    emit_av(*pp)
                    try:
                        next(filler)
                        spent += 1
                    except StopIteration:
                        pass

                # ---- normalize: evict avs to SBUF first (frees the av
                # PSUM banks for the next pg's AV accumulation), then
                # den->recip->broadcast->mul chains off SBUF. B-chain
                # (den1 at partition 0, no DMA) is emitted first so its
                # ops run while dA's DMA is in flight.
                av0e = nrm.tile([65, SC], BF16, tag="av0e", bufs=2, name="av0e")
                nc.vector.tensor_copy(av0e, av0[0:65, :])
                av1e = nrm.tile([P, SC], BF16, tag="av1e", bufs=2, name="av1e")
                nc.vector.tensor_copy(av1e, av1)
                # PE-broadcast both dens into one PSUM bank (two K=1
                # col-tiled matmuls with a constant ones stationary:
                # den0 -> partitions 0:64, den1 -> 64:128), then one
                # reciprocal covers both heads; muls are lane-aligned.
                dnp = psum.tile([P, SC], F32, tag="av", bufs=2, name="dnp")
                nc.tensor.matmul(dnp[0:64, :], lhsT=ones_sb[64:65, 0:64],
                                 rhs=av0e[64:65, :], start=True, stop=True)
                nc.tensor.matmul(dnp[64:128, :], lhsT=ones_sb[0:1, 64:128],
                                 rhs=av1e[0:1, :], start=True, stop=True)
                rc = nrm.tile([P, SC], F32, tag="rc", bufs=2, name="rc")
                nc.vector.reciprocal_approx_fast(rc, dnp)
                nc.vector.tensor_mul(
                    attnT[hp][0:64, q0:q0 + SC], av0e[0:64, :], rc[0:64, :]
                )
                nc.vector.tensor_mul(
                    attnT[hp][64:128, q0:q0 + SC], av1e[64:128, :], rc[64:128, :]
                )

            # drain remaining filler (next chunk depends on its qkrot/vnat)
            for _ in filler:
                pass
        # remaining deferred o units, then the last chunk's o_proj tail
        for g in o_gens:
            for _ in g:
                pass
        for _ in gen_o_chunk(NSC - 1, ptag="st"):
            pass

    nc.finalize()
    return nc


def _pack16(a, rows):
    """[rows*128, N] -> [128, rows*N] (row-chunk-major columns)."""
    n = a.shape[1]
    return np.ascontiguousarray(
        a.reshape(rows, P, n).transpose(1, 0, 2).reshape(P, rows * n)
    )


def prep_core_inputs(x, cos, sin, wq, wk, wv, wo, core, _shared={}):
    """Build the per-core input map (all host-side numpy)."""
    b, g = core // 4, core % 4
    S = x.shape[1]
    NHID = HID // P
    NSC = S // SC

    key = ("xTp", b, id(x))
    if key not in _shared:
        _shared.clear() if len(_shared) > 8 else None
        # [128, NHID*S] with column layout (s-chunk c, hid chunk h, s')
        xT = x[b].T.astype(NP_BF16)  # [HID, S]
        a = xT.reshape(NHID, P, NSC, SC).transpose(1, 2, 0, 3)
        _shared[key] = np.ascontiguousarray(a.reshape(P, NHID * S))
    xTp = _shared[key]

    qcols = []
    for i in range(4):
        h0, h1 = 8 * g + i, 8 * g + i + 4
        qcols.append(wq[:, h0 * D:(h0 + 1) * D])
        qcols.append(wq[:, h1 * D:(h1 + 1) * D])
    kcols = wk[:, 2 * g * D:(2 * g + 2) * D]
    vcols = wv[:, 2 * g * D:(2 * g + 2) * D]
    # m-major blocks in consumption order: k, q-pair0, v, q-pairs 1-3
    blocks = [kcols, np.concatenate(qcols[0:2], axis=1), vcols,
              np.concatenate(qcols[2:4], axis=1),
              np.concatenate(qcols[4:6], axis=1),
              np.concatenate(qcols[6:8], axis=1)]
    wqkvp = np.concatenate(
        [_pack16(b.astype(NP_BF16), NHID) for b in blocks], axis=1)
    worows = []
    for i in range(4):
        h0, h1 = 8 * g + i, 8 * g + i + 4
        worows.append(wo[h0 * D:(h0 + 1) * D, :])
        worows.append(wo[h1 * D:(h1 + 1) * D, :])
    wo_c = np.concatenate(worows, axis=0).astype(NP_BF16)
    wop = _pack16(wo_c, 4)                  # [128, 4*HID]

    cosT = np.tile(cos[:S].T, (2, 1)).astype(NP_BF16)
    sinT_h = np.concatenate([-sin[:S].T[:D // 2], sin[:S].T[D // 2:]], axis=0)
    sinT = np.tile(sinT_h, (2, 1)).astype(NP_BF16)
    trimask = np.triu(np.ones((P, P), dtype=NP_BF16))
    perm = np.zeros((P, P), dtype=NP_BF16)
    for j in range(P):
        base = (j // 64) * 64
        perm[base + ((j - base + 32) % 64), j] = 1
    onesb = np.ones((P, P), dtype=NP_BF16)
    csm = np.ascontiguousarray(
        np.concatenate([cosT, sinT, trimask, perm, onesb], axis=1)
    )

    return {"xTp": xTp, "wqkvp": wqkvp, "csm": csm, "wop": wop}


def kernel(x, cos, sin, wq, wk, wv, wo):
    x = np.asarray(x)
    S = x.shape[1]
    assert x.shape == (B, S, HID)
    if S not in _CACHE:
        _CACHE[S] = build_nc(S)
    nc = _CACHE[S]
    in_maps = [
        prep_core_inputs(x, np.asarray(cos), np.asarray(sin), np.asarray(wq),
                         np.asarray(wk), np.asarray(wv), np.asarray(wo), core)
        for core in range(8)
    ]
    res = run_bass_kernel_spmd(nc, in_maps, core_ids=list(range(8)))
    out = np.zeros((B, S, HID), np.float32)
    for core in range(8):
        out[core // 4] += res.results[core]["o_part"].astype(np.float32)
    return out



# revision 17
# speedup vs baseline: 1.0006x; 1.0006x over previous
"""GQA attention (RoPE + causal softmax + o_proj) on 8 Trainium2 NeuronCores.

Sharding: core = b*4 + g where b = batch (2), g = head-group (4).
Each core handles 8 query heads (global 8g..8g+7) and their 2 KV heads
(2g, 2g+1) for one batch element, producing a partial o_proj output
(contraction over its 512 of the 2048 hd dims). The host sums the 4
partials per batch element (o_part is bf16; host upcasts).

Per-core layout/schedule (all matmul operands bf16, fp32 PSUM accum):
  - Inputs are host-packed into 4 flat [128, N] tensors. Loads are split
    into 128-512KB pieces issued in consumption order, interleaved
    across the two HWDGE queues (sync+scalar) so the first proj matmul
    starts ~10us and the PE never starves during the HBM-bound load
    phase. csm consts + wo ride the vector (SWDGE) queue.
  - q^T/k^T built per 128-row chunk pairing heads (i, i+4); scores are
    computed transposed (S^T[k,q]) as two row-tiled K=64 matmuls.
  - AV stationary vnat[kb] = [v0(0:64) | 1 | 0*63 | v1(128:192)]:
      av0 = vnat[:,0:65].T  @ pt0 -> v0 at partitions 0:64, den0 at 64
      av1 = vnat[:,64:192].T @ pt1 -> den1 at partition 0, v1 at 64:128
    so attnT rows 64:128 are written lane-aligned and den1 feeds
    reciprocal/broadcast without a partition move.
  - AV lags scores by 3 kb (software pipeline) and the lag now carries
    ACROSS pg boundaries: pg p's last AVs drain interleaved with pg
    p+1's first scores, so the PE never waits on the ACT exp backlog at
    pg ends. Each pg's eviction/norm chain is emitted at its last AV.
  - Schedule: proj(0) dense, then attention chunk c interleaves
    proj(c+1) + o_proj(c-1) units as PE filler with exact unit counts
    and a held-back reserve drained at chunk end (covers the pg3 norm
    chain; at the last chunk it covers the gap before o_proj(3)).
  - Engine balance: exp exclusively on ACT; PSUM reads on DVE; rope
    t2+add, diag-mask half, norm muls on GPSIMD. o_part DMAs on the
    sync queue (idle after input load; keeps the exp queue clean).
"""

import numpy as np
import ml_dtypes
from contextlib import ExitStack

import concourse.mybir as mybir
from concourse import bacc
from concourse.tile import TileContext
from concourse.bass_utils import run_bass_kernel_spmd

BF16 = mybir.dt.bfloat16
F32 = mybir.dt.float32
NP_BF16 = ml_dtypes.bfloat16

HID = 2048
D = 64
H = 32           # global query heads
KV = 8           # global kv heads
B = 2
P = 128
SC = 512         # q-chunk width (also matmul free dim / PSUM bank)

_CACHE = {}


def build_nc(S):
    assert S % SC == 0
    NHID = HID // P       # hid chunks (16)
    NSB = S // P          # 128-row s-blocks
    NSC = S // SC         # 512-col s-chunks
    QCH = 4               # q chunk-pairs
    EXP = mybir.ActivationFunctionType.Exp

    nc = bacc.Bacc("TRN2", target_bir_lowering=False, debug=False)
    # host-packed flat inputs (see prep_core_inputs for layouts)
    xTp = nc.dram_tensor("xTp", [P, NHID * S], BF16, kind="ExternalInput")
    wqkvp = nc.dram_tensor("wqkvp", [P, NHID * 768], BF16, kind="ExternalInput")
    csm = nc.dram_tensor("csm", [P, 2 * S + 3 * P], BF16, kind="ExternalInput")
    wop = nc.dram_tensor("wop", [P, 4 * HID], BF16, kind="ExternalInput")
    o_part = nc.dram_tensor("o_part", [S, HID], BF16, kind="ExternalOutput")

    with TileContext(nc) as tc, ExitStack() as ctx:
        res = ctx.enter_context(tc.tile_pool(name="res", bufs=1))
        rope = ctx.enter_context(tc.tile_pool(name="rope", bufs=3))
        ptp = ctx.enter_context(tc.tile_pool(name="ptp", bufs=5))
        nrm = ctx.enter_context(tc.tile_pool(name="nrm", bufs=2))
        obp = ctx.enter_context(tc.tile_pool(name="obp", bufs=2))
        psum = ctx.enter_context(tc.tile_pool(name="psum", bufs=1, space="PSUM"))

        xtall = res.tile([P, NHID * S], BF16, tag="xtall", name="xtall")
        wqall = res.tile([P, NHID * 768], BF16, tag="wqall", name="wqall")
        csm_t = res.tile([P, 2 * S + 3 * P], BF16, tag="csm", name="csm")
        woall = res.tile([P, 4 * HID], BF16, tag="woall", name="woall")

        # ---- input DMA: consumption-ordered, paced streams.
        # HBM is the constraint during the load phase (all 8 cores pull
        # at once), and per-queue FIFO transfer order is the only
        # pacing control, so each queue carries its pieces in deadline
        # order and nothing early-needless is put in flight early (the
        # sync+scalar HWDGE queues also share one 8-slot completion-
        # semaphore pool, so issues cross-serialize past 8 in flight).
        # x s-chunk quarters alternate between the queues so the first
        # proj munit streams at 2x single-queue rate; wo (needed only
        # from ~70us) goes LAST on scalar; csm rides gpsimd SWDGE.
        def dma_piece(eng, dst, src, c0, c1):
            eng.dma_start(out=dst[:, c0:c1], in_=src[:, c0:c1])

        # weight block column ranges (block b = 16 h-chunks x 128 cols)
        def wblk(b):
            return b * NHID * P, (b + 1) * NHID * P

        # x s-chunk c column range
        def xchunk(c):
            return c * NHID * SC, (c + 1) * NHID * SC

        def xquarter(c, i):
            xa, _ = xchunk(c)
            return xa + i * 2048, xa + (i + 1) * 2048

        # x chunk-0 pieces of 1024 cols (256KB, 2 h-chunks each): even
        # pieces on sync, odd on scalar, so the h-progressive stream
        # arrives at 2x single-queue rate. b0 in 4x128KB h-ordered
        # pieces woven into the scalar stream.
        b0a, _ = wblk(0)
        xa, _ = xchunk(0)
        dma_piece(nc.scalar, wqall, wqkvp, b0a, b0a + 512)         # w h0-3
        dma_piece(nc.sync, xtall, xTp, xa, xa + 1024)              # x h0-1
        dma_piece(nc.scalar, wqall, wqkvp, b0a + 512, b0a + 1024)  # w h4-7
        dma_piece(nc.sync, xtall, xTp, xa + 1024, xa + 2048)       # x h2-3
        dma_piece(nc.scalar, xtall, xTp, xa + 2048, xa + 3072)     # x h4-5
        dma_piece(nc.sync, xtall, xTp, xa + 3072, xa + 4096)       # x h6-7
        dma_piece(nc.scalar, wqall, wqkvp, b0a + 1024, b0a + 2048)  # w h8-15
        dma_piece(nc.sync, xtall, xTp, xa + 4096, xa + 6144)       # x h8-11
        dma_piece(nc.scalar, xtall, xTp, xa + 6144, xa + 8192)     # x h12-15
        # csm on gpsimd SWDGE: consts (mask/perm/ones), then cos/sin
        nc.gpsimd.dma_start(out=csm_t[:, 0:3 * P], in_=csm[:, 0:3 * P])
        for c in range(NSC):
            base = 3 * P + c * 2 * SC
            nc.gpsimd.dma_start(out=csm_t[:, base:base + 2 * SC],
                                in_=csm[:, base:base + 2 * SC])
        # b1 (q-pair0) on sync, b2 (v) on scalar
        dma_piece(nc.sync, wqall, wqkvp, *wblk(1))
        dma_piece(nc.scalar, wqall, wqkvp, *wblk(2))
        # x s-chunk 1 quarters, alternating
        for i in range(4):
            eng = nc.sync if i % 2 == 0 else nc.scalar
            dma_piece(eng, xtall, xTp, *xquarter(1, i))
        # b3-b5 (q-pairs 1-3): sync, scalar, sync
        dma_piece(nc.sync, wqall, wqkvp, *wblk(3))
        dma_piece(nc.scalar, wqall, wqkvp, *wblk(4))
        dma_piece(nc.sync, wqall, wqkvp, *wblk(5))
        # x s-chunks 2,3: halves, alternating
        for c in (2, 3):
            xa, xb = xchunk(c)
            half = (xb - xa) // 2
            dma_piece(nc.sync, xtall, xTp, xa, xa + half)
            dma_piece(nc.scalar, xtall, xTp, xa + half, xb)
        # wo last on scalar (first consumed by o_chunk(0) during c1)
        nc.scalar.dma_start(out=woall[:, 0:2 * HID], in_=wop[:, 0:2 * HID])
        nc.scalar.dma_start(out=woall[:, 2 * HID:], in_=wop[:, 2 * HID:])

        # packed-layout views
        def xtv(h, s):      # x^T [hid chunk h, s-chunk s] -> [128, 512]
            base = (s * NHID + h) * SC
            return xtall[:, base:base + SC]

        def xtvb(h, sb):    # x^T [hid chunk h, s-block sb] -> [128, 128]
            base = ((sb // 4) * NHID + h) * SC + (sb % 4) * P
            return xtall[:, base:base + P]

        # wqall column layout: 6 blocks x 16 h x 128 cols, block order
        # (k, q-pair0, v, q-pair1, q-pair2, q-pair3) = consumption order
        _WBLK = {4: 0, 0: 1, 1: 3, 2: 4, 3: 5}

        def wqv(h, m):      # wqkv [hid chunk h, col chunk m] (m=4 -> k)
            base = (_WBLK[m] * NHID + h) * P
            return wqall[:, base:base + P]

        def wvv(h):         # wv [hid chunk h] -> [128, 128]
            base = (2 * NHID + h) * P
            return wqall[:, base:base + P]

        mask_sb = csm_t[:, 0:P]
        perm_sb = csm_t[:, P:2 * P]           # rotate-half swap
        ones_sb = csm_t[:, 2 * P:3 * P]

        def cos_v(s):       # cos cols for s-chunk s -> [128, 512]
            return csm_t[:, 3 * P + s * 2 * SC: 3 * P + s * 2 * SC + SC]

        def sin_v(s):
            return csm_t[:, 3 * P + s * 2 * SC + SC: 3 * P + (s + 1) * 2 * SC]

        def wov(i, n):      # wo [hd chunk i, hid cols n*SC..] -> [128, 512]
            return woall[:, i * HID + n * SC: i * HID + (n + 1) * SC]

        # chunks 0-3: q head pairs (i, i+4); chunk 4: k (kv0 rows 0-63, kv1 64-127)
        qkrot = []
        for m in range(5):
            t = res.tile([P, S], BF16, tag=f"qkrot{m}", name=f"qkrot{m}")
            qkrot.append(t)
        # v tiles [128, 192]: [v0(0:64) | 1 | 0*63 | v1(128:192)]
        vnat = [res.tile([P, 192], BF16, tag=f"vnat{sb}", name=f"vnat{sb}")
                for sb in range(NSB)]
        attnT = []
        for i in range(QCH):
            t = res.tile([P, S], BF16, tag=f"attnT{i}", name=f"attnT{i}")
            attnT.append(t)

        def gen_o_chunk(c, ptag="fil"):
            for qb in range(4 * c, 4 * c + 4):
                ob = obp.tile([P, HID], BF16, tag="ob", name="ob")
                for n in range(HID // SC):
                    po = psum.tile([P, SC], F32, tag=ptag, bufs=2, name="po")
                    for i in range(QCH):
                        nc.tensor.matmul(
                            po,
                            lhsT=attnT[i][:, qb * P:(qb + 1) * P],
                            rhs=wov(i, n),
                            start=(i == 0),
                            stop=(i == QCH - 1),
                        )
                    nc.vector.tensor_copy(ob[:, n * SC:(n + 1) * SC], po)
                    # per-n-slice DMA on the idle sync queue: the write
                    # starts as soon as its slice is evicted
                    nc.sync.dma_start(
                        out=o_part[qb * P:(qb + 1) * P, n * SC:(n + 1) * SC],
                        in_=ob[:, n * SC:(n + 1) * SC])
                    yield

        def gen_proj_munits(s, ms):
            """Emit projection m-chunk units for s-chunk s (yields every
            4 matmuls -> 5 yields per munit, ~0.9us filler granules).

            Rope eviction split: qraw/t1 on DVE (the PSUM readers),
            t2 + final add on GPSIMD."""
            for m in ms:
                ps = psum.tile([P, SC], F32, tag="fil", bufs=2, name="ps_proj")
                for h0 in (0, 4, 8, 12):
                    for h in range(h0, h0 + 4):
                        nc.tensor.matmul(
                            ps,
                            lhsT=wqv(h, m),
                            rhs=xtv(h, s),
                            start=(h == 0),
                            stop=(h == NHID - 1),
                        )
                    yield
                sl = slice(s * SC, (s + 1) * SC)
                # rotate_half: engines are lane-locked, so the +-32-
                # partition swap runs on the PE as a constant permutation
                # matmul (no DMA; sign lives in sinT host-side)
                qraw = rope.tile([P, SC], BF16, tag="qraw", bufs=3, name="qraw")
                nc.vector.tensor_copy(qraw, ps)
                t1 = rope.tile([P, SC], BF16, tag="t1", bufs=3, name="t1")
                nc.vector.tensor_mul(t1, ps, cos_v(s))
                ps2 = psum.tile([P, SC], F32, tag="fil", bufs=2, name="ps_rot")
                nc.tensor.matmul(ps2, lhsT=perm_sb, rhs=qraw,
                                 start=True, stop=True)
                t2 = rope.tile([P, SC], BF16, tag="t2", bufs=3, name="t2")
                nc.vector.tensor_mul(t2, ps2, sin_v(s))
                nc.gpsimd.tensor_add(qkrot[m][:, sl], t1, t2)
                yield

        def gen_v_units(s):
            for sb in range(4 * s, 4 * s + 4):
                t = vnat[sb]
                nc.gpsimd.memset(t[:, 64:65], 1.0)
                nc.gpsimd.memset(t[:, 65:128], 0.0)
                pv = psum.tile([P, 128], F32, tag="fil", bufs=2, name="ps_v")
                for h in range(8):
                    nc.tensor.matmul(
                        pv, lhsT=xtvb(h, sb), rhs=wvv(h),
                        start=(h == 0), stop=False,
                    )
                yield
                for h in range(8, NHID):
                    nc.tensor.matmul(
                        pv, lhsT=xtvb(h, sb), rhs=wvv(h),
                        start=False, stop=(h == NHID - 1),
                    )
                yield
                nc.vector.tensor_copy(t[:, 0:64], pv[:, 0:64])
                nc.vector.tensor_copy(t[:, 128:192], pv[:, 64:128])
                yield

        # filler unit counts (must match the generators above)
        N_MUNIT = 5
        N_TAIL = 3 * N_MUNIT               # 15
        N_HEAD = 2 * N_MUNIT + 4 * 3       # 22
        N_O = 16

        def gen_proj_head(s):   # pg0 deps of chunk s: k, q-pair 0, v
            yield from gen_proj_munits(s, (4, 0))
            yield from gen_v_units(s)

        def gen_proj_tail(s):   # pg1-3 deps: fillered into chunk s itself
            yield from gen_proj_munits(s, (1, 2, 3))

        def winterleave(specs):
            """Round-robin with weights: (gen, weight) pulls weight
            units per round."""
            active = [(g, w) for g, w in specs if g is not None]
            while active:
                nxt = []
                for g, w in active:
                    alive = True
                    for _ in range(w):
                        try:
                            next(g)
                        except StopIteration:
                            alive = False
                            break
                        yield
                    if alive:
                        nxt.append((g, w))
                active = nxt

        # ---- proj head of chunk 0 dense (nothing to overlap with) ----
        with nc.named_scope("projA0"):
            for _ in gen_proj_head(0):
                pass

        # deferred o-unit generators: chunk c may drain earlier chunks'
        # o units up to a per-chunk cap; the rest spill to later chunks
        o_gens = []
        # defer some o units toward chunk 3, which otherwise runs short
        # of filler (no head gen there) while ACT paces the kb loop
        o_caps = {0: 0, 1: 8, 2: 16, 3: 24}
        # filler units held back from the per-kb drip, drained at chunk
        # end to cover the pg3 norm chain (and, at the last chunk, the
        # window before o_proj(NSC-1) can start)
        o_rsrv = {0: 2, 1: 2, 2: 2, 3: 6}

        def gen_o_budget(cap):
            spent = 0
            while o_gens and spent < cap:
                try:
                    next(o_gens[0])
                except StopIteration:
                    o_gens.pop(0)
                    continue
                spent += 1
                yield

        # ---- attention chunks; proj tail(c) + head(c+1) + deferred o
        # drip into the kb loop as PE fill work while ScalarE streams
        # exps. AV matmuls lag scores by 3 kb and drain across pg
        # boundaries (pending survives the pg loop). ----
        pending = []   # (kb, pt, vs, av0, av1, first, last, hp, q0)

        def pop_pending():
            # head1's pt columns are packed at [SC : 2*SC-vs] (see the
            # scores emission) so the exp window is contiguous-valid
            kb, pt, vs, av0, av1, first, last, hp, q0 = pending.pop(0)
            nc.tensor.matmul(
                av0[0:65, vs:SC],
                lhsT=vnat[kb][:, 0:65],
                rhs=pt[:, vs:SC],
                start=first, stop=last,
            )
            nc.tensor.matmul(
                av1[:, vs:SC],
                lhsT=vnat[kb][:, 64:192],
                rhs=pt[:, SC:2 * SC - vs],
                start=first, stop=last,
            )
            if last:
                emit_norm(av0, av1, hp, q0)

        def emit_norm(av0, av1, hp, q0):
            # evict avs to SBUF (frees the av PSUM banks), then
            # den->recip->broadcast->mul chains off SBUF.
            av0e = nrm.tile([65, SC], BF16, tag="av0e", bufs=2, name="av0e")
            nc.vector.tensor_copy(av0e, av0[0:65, :])
            av1e = nrm.tile([P, SC], BF16, tag="av1e", bufs=2, name="av1e")
            nc.vector.tensor_copy(av1e, av1)
            # PE-broadcast both dens into one PSUM bank (two K=1
            # col-tiled matmuls with a constant ones stationary:
            # den0 -> partitions 0:64, den1 -> 64:128), then one
            # reciprocal covers both heads; muls are lane-aligned.
            # Tag "fil" (not "av"): with the cross-pg AV pipeline the
            # next pg's av tiles claim the freed av slots first, so a
            # dnp on tag av would stall a full pg behind them.
            dnp = psum.tile([P, SC], F32, tag="fil", bufs=2, name="dnp")
            nc.tensor.matmul(dnp[0:64, :], lhsT=ones_sb[64:65, 0:64],
                             rhs=av0e[64:65, :], start=True, stop=True)
            nc.tensor.matmul(dnp[64:128, :], lhsT=ones_sb[0:1, 64:128],
                             rhs=av1e[0:1, :], start=True, stop=True)
            rc = nrm.tile([P, SC], F32, tag="rc", bufs=2, name="rc")
            nc.vector.reciprocal_approx_fast(rc, dnp)
            nc.vector.tensor_mul(
                attnT[hp][0:64, q0:q0 + SC], av0e[0:64, :], rc[0:64, :]
            )
            nc.vector.tensor_mul(
                attnT[hp][64:128, q0:q0 + SC], av1e[64:128, :], rc[64:128, :]
            )

        for c in range(NSC):
          with nc.named_scope(f"attn_c{c}"):
            q0 = c * SC
            nkb = 4 * c + 4
            if c >= 1:
                o_gens.append(gen_o_chunk(c - 1))
            avail_o = min(o_caps[c], N_O * len(o_gens))
            tail_g = [gen_proj_tail(c)]
            tail_pulled = [0]
            filler = winterleave([
                (gen_proj_head(c + 1) if c + 1 < NSC else None, 1),
                (gen_o_budget(o_caps[c]), 1),
            ])

            def pull_tail():
                if not tail_g:
                    return False
                try:
                    next(tail_g[0])
                    tail_pulled[0] += 1
                    return True
                except StopIteration:
                    tail_g.clear()
                    return False

            seq = [0]

            def pull_one():
                """One filler unit; tail-weighted 2:1 like the baseline
                (its rope chain latency must stay well ahead of the pg
                score deadlines)."""
                seq[0] += 1
                if seq[0] % 3 != 0 and pull_tail():
                    return True
                try:
                    next(filler)
                    return True
                except StopIteration:
                    return pull_tail()

            n_units = (N_TAIL + (N_HEAD if c + 1 < NSC else 0)
                       + avail_o - o_rsrv[c])
            total_iters = 4 * nkb
            LEAD = 5
            it = 0
            spent = 0
            for pg in (0, 1, 2, 3):
                av0 = psum.tile([P, SC], F32, tag="av", bufs=2, name="av0")
                av1 = psum.tile([P, SC], F32, tag="av", bufs=2, name="av1")
                hp = pg
                kb_order = list(range(4 * c, nkb)) + list(range(0, 4 * c))
                first_kb, last_kb = kb_order[0], kb_order[-1]
                for kb in kb_order:
                    # lookahead deadline: tail munit m must be fully
                    # emitted LEAD iterations before pg m's first score
                    # so its DVE/GPSIMD rope chain completes in time
                    need_m = min(3, (it + LEAD) // nkb)
                    while tail_g and tail_pulled[0] < N_MUNIT * need_m:
                        if pull_tail():
                            spent += 1
                    vs = max(0, (kb - 4 * c) * P)  # first valid col in chunk
                    st = psum.tile([P, 2 * SC], F32, tag="st", bufs=2, name="st")
                    nc.tensor.matmul(
                        st[:, vs:SC],
                        lhsT=qkrot[4][0:64, kb * P:(kb + 1) * P],
                        rhs=qkrot[hp][0:64, q0 + vs:q0 + SC],
                        start=True, stop=True,
                    )
                    # head1 packed at [SC : 2*SC-vs]: keeps the exp
                    # window [vs : 2*SC-vs] contiguous and all-valid
                    nc.tensor.matmul(
                        st[:, SC:2 * SC - vs],
                        lhsT=qkrot[4][64:128, kb * P:(kb + 1) * P],
                        rhs=qkrot[hp][64:128, q0 + vs:q0 + SC],
                        start=True, stop=True,
                    )
                    if len(pending) >= 3:
                        pop_pending()
                    pt = ptp.tile([P, 2 * SC], BF16, tag="pt", name="pt")
                    nc.scalar.activation(
                        pt[:, vs:2 * SC - vs], st[:, vs:2 * SC - vs],
                        EXP, scale=0.125
                    )
                    if kb - 4 * c >= 0:  # diagonal block: mask triangle
                        nc.vector.tensor_mul(
                            pt[:, vs:vs + P], pt[:, vs:vs + P], mask_sb
                        )
                        nc.gpsimd.tensor_mul(
                            pt[:, SC:SC + P],
                            pt[:, SC:SC + P], mask_sb
                        )
                    pending.append((kb, pt, vs, av0, av1,
                                    kb == first_kb, kb == last_kb, hp, q0))
                    it += 1
                    want = (it * n_units) // total_iters
                    while spent < want:
                        if pull_one():
                            spent += 1
                        else:
                            spent = want
                            break

            # flush the AV pipeline at chunk end (norm(pg3) must land
            # before the next chunk's o_proj filler reads attnT),
            # interleaved with the held-back reserve filler so the PE
            # stays fed while ACT drains the exp backlog
            while pending:
                pop_pending()
                pull_one()
            # drain remaining filler (next chunk depends on its qkrot/vnat)
            while pull_one():
                pass
        # remaining deferred o units, then the last chunk's o_proj tail
        for g in o_gens:
            for _ in g:
                pass
        for _ in gen_o_chunk(NSC - 1, ptag="st"):
            pass

    nc.finalize()
    return nc


def _pack16(a, rows):
    """[rows*128, N] -> [128, rows*N] (row-chunk-major columns)."""
    n = a.shape[1]
    return np.ascontiguousarray(
        a.reshape(rows, P, n).transpose(1, 0, 2).reshape(P, rows * n)
    )


def prep_core_inputs(x, cos, sin, wq, wk, wv, wo, core, _shared={}):
    """Build the per-core input map (all host-side numpy)."""
    b, g = core // 4, core % 4
    S = x.shape[1]
    NHID = HID // P
    NSC = S // SC

    key = ("xTp", b, id(x))
    if key not in _shared:
        _shared.clear() if len(_shared) > 8 else None
        # [128, NHID*S] with column layout (s-chunk c, hid chunk h, s')
        xT = x[b].T.astype(NP_BF16)  # [HID, S]
        a = xT.reshape(NHID, P, NSC, SC).transpose(1, 2, 0, 3)
        _shared[key] = np.ascontiguousarray(a.reshape(P, NHID * S))
    xTp = _shared[key]

    qcols = []
    for i in range(4):
        h0, h1 = 8 * g + i, 8 * g + i + 4
        qcols.append(wq[:, h0 * D:(h0 + 1) * D])
        qcols.append(wq[:, h1 * D:(h1 + 1) * D])
    kcols = wk[:, 2 * g * D:(2 * g + 2) * D]
    vcols = wv[:, 2 * g * D:(2 * g + 2) * D]
    # m-major blocks in consumption order: k, q-pair0, v, q-pairs 1-3
    blocks = [kcols, np.concatenate(qcols[0:2], axis=1), vcols,
              np.concatenate(qcols[2:4], axis=1),
              np.concatenate(qcols[4:6], axis=1),
              np.concatenate(qcols[6:8], axis=1)]
    wqkvp = np.concatenate(
        [_pack16(b.astype(NP_BF16), NHID) for b in blocks], axis=1)
    worows = []
    for i in range(4):
        h0, h1 = 8 * g + i, 8 * g + i + 4
        worows.append(wo[h0 * D:(h0 + 1) * D, :])
        worows.append(wo[h1 * D:(h1 + 1) * D, :])
    wo_c = np.concatenate(worows, axis=0).astype(NP_BF16)
    wop = _pack16(wo_c, 4)                  # [128, 4*HID]

    # csm layout: [mask | perm | ones | (cos_c | sin_c) per s-chunk]
    cosT = np.tile(cos[:S].T, (2, 1)).astype(NP_BF16)   # [128, S]
    sinT_h = np.concatenate([-sin[:S].T[:D // 2], sin[:S].T[D // 2:]], axis=0)
    sinT = np.tile(sinT_h, (2, 1)).astype(NP_BF16)
    trimask = np.triu(np.ones((P, P), dtype=NP_BF16))
    perm = np.zeros((P, P), dtype=NP_BF16)
    for j in range(P):
        base = (j // 64) * 64
        perm[base + ((j - base + 32) % 64), j] = 1
    onesb = np.ones((P, P), dtype=NP_BF16)
    cs_blocks = []
    for c in range(NSC):
        cs_blocks.append(cosT[:, c * SC:(c + 1) * SC])
        cs_blocks.append(sinT[:, c * SC:(c + 1) * SC])
    csm = np.ascontiguousarray(
        np.concatenate([trimask, perm, onesb] + cs_blocks, axis=1)
    )

    return {"xTp": xTp, "wqkvp": wqkvp, "csm": csm, "wop": wop}


def kernel(x, cos, sin, wq, wk, wv, wo):
    x = np.asarray(x)
    S = x.shape[1]
    assert x.shape == (B, S, HID)
    if S not in _CACHE:
        _CACHE[S] = build_nc(S)
    nc = _CACHE[S]
    in_maps = [
        prep_core_inputs(x, np.asarray(cos), np.asarray(sin), np.asarray(wq),
                         np.asarray(wk), np.asarray(wv), np.asarray(wo), core)
        for core in range(8)
    ]
    res = run_bass_kernel_spmd(nc, in_maps, core_ids=list(range(8)))
    out = np.zeros((B, S, HID), np.float32)
    for core in range(8):
        out[core // 4] += res.results[core]["o_part"].astype(np.float32)
    return out


# revision 20
# speedup vs baseline: 1.0146x; 1.0140x over previous
"""GQA attention (RoPE + causal softmax + o_proj) on 8 Trainium2 NeuronCores.

Sharding: core = b*4 + g where b = batch (2), g = head-group (4).
Each core handles 8 query heads (global 8g..8g+7) and their 2 KV heads
(2g, 2g+1) for one batch element, producing a partial o_proj output
(contraction over its 512 of the 2048 hd dims). The host sums the 4
partials per batch element (o_part is bf16; host upcasts).

Per-core layout/schedule (all matmul operands bf16, fp32 PSUM accum):
  - Inputs are host-packed into 4 flat [128, N] tensors. Loads are split
    into 128-512KB pieces issued in consumption order, interleaved
    across the two HWDGE queues (sync+scalar) so the first proj matmul
    starts ~10us and the PE never starves during the HBM-bound load
    phase. csm consts + wo ride the vector (SWDGE) queue.
  - q^T/k^T built per 128-row chunk pairing heads (i, i+4); scores are
    computed transposed (S^T[k,q]) as two row-tiled K=64 matmuls.
  - AV stationary vnat[kb] = [v0(0:64) | 1 | 0*63 | v1(128:192)]:
      av0 = vnat[:,0:65].T  @ pt0 -> v0 at partitions 0:64, den0 at 64
      av1 = vnat[:,64:192].T @ pt1 -> den1 at partition 0, v1 at 64:128
    so attnT rows 64:128 are written lane-aligned and den1 feeds
    reciprocal/broadcast without a partition move.
  - AV lags scores by 3 kb (software pipeline) and the lag now carries
    ACROSS pg boundaries: pg p's last AVs drain interleaved with pg
    p+1's first scores, so the PE never waits on the ACT exp backlog at
    pg ends. Each pg's eviction/norm chain is emitted at its last AV.
  - Schedule: proj(0) dense, then attention chunk c interleaves
    proj(c+1) + o_proj(c-1) units as PE filler with exact unit counts
    and a held-back reserve drained at chunk end (covers the pg3 norm
    chain; at the last chunk it covers the gap before o_proj(3)).
  - Engine balance: exp exclusively on ACT; PSUM reads on DVE; rope
    t2+add, diag-mask half, norm muls on GPSIMD. o_part DMAs on the
    sync queue (idle after input load; keeps the exp queue clean).
"""

import numpy as np
import ml_dtypes
from contextlib import ExitStack

import concourse.mybir as mybir
from concourse import bacc
from concourse.tile import TileContext
from concourse.bass_utils import run_bass_kernel_spmd

BF16 = mybir.dt.bfloat16
F32 = mybir.dt.float32
NP_BF16 = ml_dtypes.bfloat16

HID = 2048
D = 64
H = 32           # global query heads
KV = 8           # global kv heads
B = 2
P = 128
SC = 512         # q-chunk width (also matmul free dim / PSUM bank)

_CACHE = {}


def build_nc(S):
    assert S % SC == 0
    NHID = HID // P       # hid chunks (16)
    NSB = S // P          # 128-row s-blocks
    NSC = S // SC         # 512-col s-chunks
    QCH = 4               # q chunk-pairs
    EXP = mybir.ActivationFunctionType.Exp

    nc = bacc.Bacc("TRN2", target_bir_lowering=False, debug=False)
    # host-packed flat inputs (see prep_core_inputs for layouts)
    xTp = nc.dram_tensor("xTp", [P, NHID * S], BF16, kind="ExternalInput")
    wqkvp = nc.dram_tensor("wqkvp", [P, NHID * 768], BF16, kind="ExternalInput")
    csm = nc.dram_tensor("csm", [P, 2 * S + 3 * P], BF16, kind="ExternalInput")
    wop = nc.dram_tensor("wop", [P, 4 * HID], BF16, kind="ExternalInput")
    o_part = nc.dram_tensor("o_part", [S, HID], BF16, kind="ExternalOutput")

    with TileContext(nc) as tc, ExitStack() as ctx:
        res = ctx.enter_context(tc.tile_pool(name="res", bufs=1))
        rope = ctx.enter_context(tc.tile_pool(name="rope", bufs=3))
        ptp = ctx.enter_context(tc.tile_pool(name="ptp", bufs=5))
        nrm = ctx.enter_context(tc.tile_pool(name="nrm", bufs=2))
        obp = ctx.enter_context(tc.tile_pool(name="obp", bufs=2))
        psum = ctx.enter_context(tc.tile_pool(name="psum", bufs=1, space="PSUM"))

        xtall = res.tile([P, NHID * S], BF16, tag="xtall", name="xtall")
        wqall = res.tile([P, NHID * 768], BF16, tag="wqall", name="wqall")
        csm_t = res.tile([P, 2 * S + 3 * P], BF16, tag="csm", name="csm")
        woall = res.tile([P, 4 * HID], BF16, tag="woall", name="woall")

        # ---- input DMA: consumption-ordered, paced streams.
        # HBM is the constraint during the load phase (all 8 cores pull
        # at once), and per-queue FIFO transfer order is the only
        # pacing control, so each queue carries its pieces in deadline
        # order and nothing early-needless is put in flight early (the
        # sync+scalar HWDGE queues also share one 8-slot completion-
        # semaphore pool, so issues cross-serialize past 8 in flight).
        # x s-chunk quarters alternate between the queues so the first
        # proj munit streams at 2x single-queue rate; wo (needed only
        # from ~70us) goes LAST on scalar; csm rides gpsimd SWDGE.
        def dma_piece(eng, dst, src, c0, c1):
            eng.dma_start(out=dst[:, c0:c1], in_=src[:, c0:c1])

        # weight block column ranges (block b = 16 h-chunks x 128 cols)
        def wblk(b):
            return b * NHID * P, (b + 1) * NHID * P

        # x s-chunk c column range
        def xchunk(c):
            return c * NHID * SC, (c + 1) * NHID * SC

        def xquarter(c, i):
            xa, _ = xchunk(c)
            return xa + i * 2048, xa + (i + 1) * 2048

        # x chunk-0 pieces of 1024 cols (256KB, 2 h-chunks each) spread
        # across sync + gpsimd so the h-progressive stream rides two
        # queues; the weights stream (b0 first, gating LDWEIGHTS) rides
        # scalar. csm consts lead the gpsimd queue (needed by the first
        # rope); cos/sin chunks follow the early x pieces there.
        b0a, _ = wblk(0)
        xa, _ = xchunk(0)
        nc.gpsimd.dma_start(out=csm_t[:, 0:3 * P], in_=csm[:, 0:3 * P])
        dma_piece(nc.scalar, wqall, wqkvp, b0a, b0a + 1024)        # w h0-7
        dma_piece(nc.sync, xtall, xTp, xa, xa + 1024)              # x h0-1
        dma_piece(nc.gpsimd, xtall, xTp, xa + 2048, xa + 3072)     # x h4-5
        dma_piece(nc.scalar, wqall, wqkvp, b0a + 1024, b0a + 2048)  # w h8-15
        dma_piece(nc.sync, xtall, xTp, xa + 1024, xa + 2048)       # x h2-3
        dma_piece(nc.gpsimd, xtall, xTp, xa + 4096, xa + 5120)     # x h8-9
        dma_piece(nc.sync, xtall, xTp, xa + 3072, xa + 4096)       # x h6-7
        dma_piece(nc.gpsimd, xtall, xTp, xa + 6144, xa + 7168)     # x h12-13
        dma_piece(nc.sync, xtall, xTp, xa + 5120, xa + 6144)       # x h10-11
        dma_piece(nc.sync, xtall, xTp, xa + 7168, xa + 8192)       # x h14-15
        # b1 (q-pair0), b2 (v) follow on scalar in consumption order
        dma_piece(nc.scalar, wqall, wqkvp, *wblk(1))
        dma_piece(nc.scalar, wqall, wqkvp, *wblk(2))
        # cos/sin chunks on gpsimd (consumed by DVE rope muls)
        for c in range(NSC):
            base = 3 * P + c * 2 * SC
            nc.gpsimd.dma_start(out=csm_t[:, base:base + 2 * SC],
                                in_=csm[:, base:base + 2 * SC])
        # x s-chunk 1 quarters, alternating
        for i in range(4):
            eng = nc.sync if i % 2 == 0 else nc.scalar
            dma_piece(eng, xtall, xTp, *xquarter(1, i))
        # b3-b5 (q-pairs 1-3): sync, scalar, sync
        dma_piece(nc.sync, wqall, wqkvp, *wblk(3))
        dma_piece(nc.scalar, wqall, wqkvp, *wblk(4))
        dma_piece(nc.sync, wqall, wqkvp, *wblk(5))
        # x s-chunks 2,3: halves, alternating
        for c in (2, 3):
            xa, xb = xchunk(c)
            half = (xb - xa) // 2
            dma_piece(nc.sync, xtall, xTp, xa, xa + half)
            dma_piece(nc.scalar, xtall, xTp, xa + half, xb)
        # wo last on scalar (first consumed by o_chunk(0) during c1)
        nc.scalar.dma_start(out=woall[:, 0:2 * HID], in_=wop[:, 0:2 * HID])
        nc.scalar.dma_start(out=woall[:, 2 * HID:], in_=wop[:, 2 * HID:])

        # packed-layout views
        def xtv(h, s):      # x^T [hid chunk h, s-chunk s] -> [128, 512]
            base = (s * NHID + h) * SC
            return xtall[:, base:base + SC]

        def xtvb(h, sb):    # x^T [hid chunk h, s-block sb] -> [128, 128]
            base = ((sb // 4) * NHID + h) * SC + (sb % 4) * P
            return xtall[:, base:base + P]

        # wqall column layout: 6 blocks x 16 h x 128 cols, block order
        # (k, q-pair0, v, q-pair1, q-pair2, q-pair3) = consumption order
        _WBLK = {4: 0, 0: 1, 1: 3, 2: 4, 3: 5}

        def wqv(h, m):      # wqkv [hid chunk h, col chunk m] (m=4 -> k)
            base = (_WBLK[m] * NHID + h) * P
            return wqall[:, base:base + P]

        def wvv(h):         # wv [hid chunk h] -> [128, 128]
            base = (2 * NHID + h) * P
            return wqall[:, base:base + P]

        mask_sb = csm_t[:, 0:P]
        perm_sb = csm_t[:, P:2 * P]           # rotate-half swap
        ones_sb = csm_t[:, 2 * P:3 * P]

        def cos_v(s):       # cos cols for s-chunk s -> [128, 512]
            return csm_t[:, 3 * P + s * 2 * SC: 3 * P + s * 2 * SC + SC]

        def sin_v(s):
            return csm_t[:, 3 * P + s * 2 * SC + SC: 3 * P + (s + 1) * 2 * SC]

        def wov(i, n):      # wo [hd chunk i, hid cols n*SC..] -> [128, 512]
            return woall[:, i * HID + n * SC: i * HID + (n + 1) * SC]

        # chunks 0-3: q head pairs (i, i+4); chunk 4: k (kv0 rows 0-63, kv1 64-127)
        qkrot = []
        for m in range(5):
            t = res.tile([P, S], BF16, tag=f"qkrot{m}", name=f"qkrot{m}")
            qkrot.append(t)
        # v tiles [128, 192]: [v0(0:64) | 1 | 0*63 | v1(128:192)]
        vnat = [res.tile([P, 192], BF16, tag=f"vnat{sb}", name=f"vnat{sb}")
                for sb in range(NSB)]
        attnT = []
        for i in range(QCH):
            t = res.tile([P, S], BF16, tag=f"attnT{i}", name=f"attnT{i}")
            attnT.append(t)

        def gen_o_chunk(c, ptag="fil"):
            for qb in range(4 * c, 4 * c + 4):
                ob = obp.tile([P, HID], BF16, tag="ob", name="ob")
                for n in range(HID // SC):
                    po = psum.tile([P, SC], F32, tag=ptag, bufs=2, name="po")
                    for i in range(QCH):
                        nc.tensor.matmul(
                            po,
                            lhsT=attnT[i][:, qb * P:(qb + 1) * P],
                            rhs=wov(i, n),
                            start=(i == 0),
                            stop=(i == QCH - 1),
                        )
                    nc.vector.tensor_copy(ob[:, n * SC:(n + 1) * SC], po)
                    # per-n-slice DMA on the idle sync queue: the write
                    # starts as soon as its slice is evicted
                    nc.sync.dma_start(
                        out=o_part[qb * P:(qb + 1) * P, n * SC:(n + 1) * SC],
                        in_=ob[:, n * SC:(n + 1) * SC])
                    yield

        def gen_proj_munits(s, ms):
            """Emit projection m-chunk units for s-chunk s (yields every
            4 matmuls -> 5 yields per munit, ~0.9us filler granules).

            Rope eviction split: qraw/t1 on DVE (the PSUM readers),
            t2 + final add on GPSIMD."""
            for m in ms:
                ps = psum.tile([P, SC], F32, tag="fil", bufs=2, name="ps_proj")
                for h0 in (0, 4, 8, 12):
                    for h in range(h0, h0 + 4):
                        nc.tensor.matmul(
                            ps,
                            lhsT=wqv(h, m),
                            rhs=xtv(h, s),
                            start=(h == 0),
                            stop=(h == NHID - 1),
                        )
                    yield
                sl = slice(s * SC, (s + 1) * SC)
                # rotate_half: engines are lane-locked, so the +-32-
                # partition swap runs on the PE as a constant permutation
                # matmul (no DMA; sign lives in sinT host-side)
                qraw = rope.tile([P, SC], BF16, tag="qraw", bufs=3, name="qraw")
                nc.vector.tensor_copy(qraw, ps)
                t1 = rope.tile([P, SC], BF16, tag="t1", bufs=3, name="t1")
                nc.vector.tensor_mul(t1, ps, cos_v(s))
                ps2 = psum.tile([P, SC], F32, tag="fil", bufs=2, name="ps_rot")
                nc.tensor.matmul(ps2, lhsT=perm_sb, rhs=qraw,
                                 start=True, stop=True)
                t2 = rope.tile([P, SC], BF16, tag="t2", bufs=3, name="t2")
                nc.vector.tensor_mul(t2, ps2, sin_v(s))
                nc.gpsimd.tensor_add(qkrot[m][:, sl], t1, t2)
                yield

        def gen_v_units(s):
            for sb in range(4 * s, 4 * s + 4):
                t = vnat[sb]
                nc.gpsimd.memset(t[:, 64:65], 1.0)
                nc.gpsimd.memset(t[:, 65:128], 0.0)
                pv = psum.tile([P, 128], F32, tag="fil", bufs=2, name="ps_v")
                for h in range(8):
                    nc.tensor.matmul(
                        pv, lhsT=xtvb(h, sb), rhs=wvv(h),
                        start=(h == 0), stop=False,
                    )
                yield
                for h in range(8, NHID):
                    nc.tensor.matmul(
                        pv, lhsT=xtvb(h, sb), rhs=wvv(h),
                        start=False, stop=(h == NHID - 1),
                    )
                yield
                nc.vector.tensor_copy(t[:, 0:64], pv[:, 0:64])
                nc.vector.tensor_copy(t[:, 128:192], pv[:, 64:128])
                yield

        # filler unit counts (must match the generators above)
        N_MUNIT = 5
        N_TAIL = 3 * N_MUNIT               # 15
        N_HEAD = 2 * N_MUNIT + 4 * 3       # 22
        N_O = 16

        def gen_proj_head(s):   # pg0 deps of chunk s: k, q-pair 0, v
            yield from gen_proj_munits(s, (4, 0))
            yield from gen_v_units(s)

        def gen_proj_tail(s):   # pg1-3 deps: fillered into chunk s itself
            yield from gen_proj_munits(s, (1, 2, 3))

        def winterleave(specs):
            """Round-robin with weights: (gen, weight) pulls weight
            units per round."""
            active = [(g, w) for g, w in specs if g is not None]
            while active:
                nxt = []
                for g, w in active:
                    alive = True
                    for _ in range(w):
                        try:
                            next(g)
                        except StopIteration:
                            alive = False
                            break
                        yield
                    if alive:
                        nxt.append((g, w))
                active = nxt

        # ---- proj head of chunk 0 dense (nothing to overlap with) ----
        with nc.named_scope("projA0"):
            for _ in gen_proj_head(0):
                pass

        # deferred o-unit generators: chunk c may drain earlier chunks'
        # o units up to a per-chunk cap; the rest spill to later chunks
        o_gens = []
        # defer some o units toward chunk 3, which otherwise runs short
        # of filler (no head gen there) while ACT paces the kb loop
        o_caps = {0: 0, 1: 8, 2: 16, 3: 24}
        # filler units held back from the per-kb drip, drained at chunk
        # end to cover the pg3 norm chain (and, at the last chunk, the
        # window before o_proj(NSC-1) can start)
        o_rsrv = {0: 2, 1: 4, 2: 4, 3: 6}

        def gen_o_budget(cap):
            spent = 0
            while o_gens and spent < cap:
                try:
                    next(o_gens[0])
                except StopIteration:
                    o_gens.pop(0)
                    continue
                spent += 1
                yield

        # ---- attention chunks; proj tail(c) + head(c+1) + deferred o
        # drip into the kb loop as PE fill work while ScalarE streams
        # exps. AV matmuls lag scores by 3 kb and drain across pg
        # boundaries (pending survives the pg loop). ----
        pending = []   # (kb, pt, vs, av0, av1, first, last, hp, q0)

        def pop_pending():
            # head1's pt columns are packed at [SC : 2*SC-vs] (see the
            # scores emission) so the exp window is contiguous-valid
            kb, pt, vs, av0, av1, first, last, hp, q0 = pending.pop(0)
            nc.tensor.matmul(
                av0[0:65, vs:SC],
                lhsT=vnat[kb][:, 0:65],
                rhs=pt[:, vs:SC],
                start=first, stop=last,
            )
            nc.tensor.matmul(
                av1[:, vs:SC],
                lhsT=vnat[kb][:, 64:192],
                rhs=pt[:, SC:2 * SC - vs],
                start=first, stop=last,
            )
            if last:
                emit_norm(av0, av1, hp, q0)

        def emit_norm(av0, av1, hp, q0):
            # evict avs to SBUF (frees the av PSUM banks), then
            # den->recip->broadcast->mul chains off SBUF.
            av0e = nrm.tile([65, SC], BF16, tag="av0e", bufs=2, name="av0e")
            nc.vector.tensor_copy(av0e, av0[0:65, :])
            av1e = nrm.tile([P, SC], BF16, tag="av1e", bufs=2, name="av1e")
            nc.vector.tensor_copy(av1e, av1)
            # PE-broadcast both dens into one PSUM bank (two K=1
            # col-tiled matmuls with a constant ones stationary:
            # den0 -> partitions 0:64, den1 -> 64:128), then one
            # reciprocal covers both heads; muls are lane-aligned.
            # Tag "fil" (not "av"): with the cross-pg AV pipeline the
            # next pg's av tiles claim the freed av slots first, so a
            # dnp on tag av would stall a full pg behind them.
            dnp = psum.tile([P, SC], F32, tag="fil", bufs=2, name="dnp")
            nc.tensor.matmul(dnp[0:64, :], lhsT=ones_sb[64:65, 0:64],
                             rhs=av0e[64:65, :], start=True, stop=True)
            nc.tensor.matmul(dnp[64:128, :], lhsT=ones_sb[0:1, 64:128],
                             rhs=av1e[0:1, :], start=True, stop=True)
            rc = nrm.tile([P, SC], F32, tag="rc", bufs=2, name="rc")
            nc.vector.reciprocal_approx_fast(rc, dnp)
            nc.vector.tensor_mul(
                attnT[hp][0:64, q0:q0 + SC], av0e[0:64, :], rc[0:64, :]
            )
            nc.vector.tensor_mul(
                attnT[hp][64:128, q0:q0 + SC], av1e[64:128, :], rc[64:128, :]
            )

        for c in range(NSC):
          with nc.named_scope(f"attn_c{c}"):
            q0 = c * SC
            nkb = 4 * c + 4
            if c >= 1:
                o_gens.append(gen_o_chunk(c - 1))
            avail_o = min(o_caps[c], N_O * len(o_gens))
            tail_g = [gen_proj_tail(c)]
            tail_pulled = [0]
            filler = winterleave([
                (gen_proj_head(c + 1) if c + 1 < NSC else None, 1),
                (gen_o_budget(o_caps[c]), 1),
            ])

            def pull_tail():
                if not tail_g:
                    return False
                try:
                    next(tail_g[0])
                    tail_pulled[0] += 1
                    return True
                except StopIteration:
                    tail_g.clear()
                    return False

            seq = [0]

            def pull_one():
                """One filler unit; tail-weighted 2:1 like the baseline
                (its rope chain latency must stay well ahead of the pg
                score deadlines)."""
                seq[0] += 1
                if seq[0] % 3 != 0 and pull_tail():
                    return True
                try:
                    next(filler)
                    return True
                except StopIteration:
                    return pull_tail()

            n_units = (N_TAIL + (N_HEAD if c + 1 < NSC else 0)
                       + avail_o - o_rsrv[c])
            total_iters = 4 * nkb
            LEAD = 7
            it = 0
            spent = 0
            for pg in (0, 1, 2, 3):
                av0 = psum.tile([P, SC], F32, tag="av", bufs=2, name="av0")
                av1 = psum.tile([P, SC], F32, tag="av", bufs=2, name="av1")
                hp = pg
                kb_order = list(range(4 * c, nkb)) + list(range(0, 4 * c))
                first_kb, last_kb = kb_order[0], kb_order[-1]
                for kb in kb_order:
                    # lookahead deadline: tail munit m must be fully
                    # emitted LEAD iterations before pg m's first score
                    # so its DVE/GPSIMD rope chain completes in time
                    need_m = min(3, (it + LEAD) // nkb)
                    while tail_g and tail_pulled[0] < N_MUNIT * need_m:
                        if pull_tail():
                            spent += 1
                    vs = max(0, (kb - 4 * c) * P)  # first valid col in chunk
                    st = psum.tile([P, 2 * SC], F32, tag="st", bufs=2, name="st")
                    nc.tensor.matmul(
                        st[:, vs:SC],
                        lhsT=qkrot[4][0:64, kb * P:(kb + 1) * P],
                        rhs=qkrot[hp][0:64, q0 + vs:q0 + SC],
                        start=True, stop=True,
                    )
                    # head1 packed at [SC : 2*SC-vs]: keeps the exp
                    # window [vs : 2*SC-vs] contiguous and all-valid
                    nc.tensor.matmul(
                        st[:, SC:2 * SC - vs],
                        lhsT=qkrot[4][64:128, kb * P:(kb + 1) * P],
                        rhs=qkrot[hp][64:128, q0 + vs:q0 + SC],
                        start=True, stop=True,
                    )
                    if len(pending) >= 3:
                        pop_pending()
                    pt = ptp.tile([P, 2 * SC], BF16, tag="pt", name="pt")
                    nc.scalar.activation(
                        pt[:, vs:2 * SC - vs], st[:, vs:2 * SC - vs],
                        EXP, scale=0.125
                    )
                    if kb - 4 * c >= 0:  # diagonal block: mask triangle
                        nc.vector.tensor_mul(
                            pt[:, vs:vs + P], pt[:, vs:vs + P], mask_sb
                        )
                        nc.gpsimd.tensor_mul(
                            pt[:, SC:SC + P],
                            pt[:, SC:SC + P], mask_sb
                        )
                    pending.append((kb, pt, vs, av0, av1,
                                    kb == first_kb, kb == last_kb, hp, q0))
                    it += 1
                    want = (it * n_units) // total_iters
                    while spent < want:
                        if pull_one():
                            spent += 1
                        else:
                            spent = want
                            break

            # flush the AV pipeline at chunk end (norm(pg3) must land
            # before the next chunk's o_proj filler reads attnT),
            # interleaved with the held-back reserve filler so the PE
            # stays fed while ACT drains the exp backlog
            while pending:
                pop_pending()
                pull_one()
            # drain remaining filler (next chunk depends on its qkrot/vnat)
            while pull_one():
                pass
        # remaining deferred o units, then the last chunk's o_proj tail
        for g in o_gens:
            for _ in g:
                pass
        for _ in gen_o_chunk(NSC - 1, ptag="st"):
            pass

    nc.finalize()
    return nc


def _pack16(a, rows):
    """[rows*128, N] -> [128, rows*N] (row-chunk-major columns)."""
    n = a.shape[1]
    return np.ascontiguousarray(
        a.reshape(rows, P, n).transpose(1, 0, 2).reshape(P, rows * n)
    )


def prep_core_inputs(x, cos, sin, wq, wk, wv, wo, core, _shared={}):
    """Build the per-core input map (all host-side numpy)."""
    b, g = core // 4, core % 4
    S = x.shape[1]
    NHID = HID // P
    NSC = S // SC

    key = ("xTp", b, id(x))
    if key not in _shared:
        _shared.clear() if len(_shared) > 8 else None
        # [128, NHID*S] with column layout (s-chunk c, hid chunk h, s')
        xT = x[b].T.astype(NP_BF16)  # [HID, S]
        a = xT.reshape(NHID, P, NSC, SC).transpose(1, 2, 0, 3)
        _shared[key] = np.ascontiguousarray(a.reshape(P, NHID * S))
    xTp = _shared[key]

    qcols = []
    for i in range(4):
        h0, h1 = 8 * g + i, 8 * g + i + 4
        qcols.append(wq[:, h0 * D:(h0 + 1) * D])
        qcols.append(wq[:, h1 * D:(h1 + 1) * D])
    kcols = wk[:, 2 * g * D:(2 * g + 2) * D]
    vcols = wv[:, 2 * g * D:(2 * g + 2) * D]
    # m-major blocks in consumption order: k, q-pair0, v, q-pairs 1-3
    blocks = [kcols, np.concatenate(qcols[0:2], axis=1), vcols,
              np.concatenate(qcols[2:4], axis=1),
              np.concatenate(qcols[4:6], axis=1),
              np.concatenate(qcols[6:8], axis=1)]
    wqkvp = np.concatenate(
        [_pack16(b.astype(NP_BF16), NHID) for b in blocks], axis=1)
    worows = []
    for i in range(4):
        h0, h1 = 8 * g + i, 8 * g + i + 4
        worows.append(wo[h0 * D:(h0 + 1) * D, :])
        worows.append(wo[h1 * D:(h1 + 1) * D, :])
    wo_c = np.concatenate(worows, axis=0).astype(NP_BF16)
    wop = _pack16(wo_c, 4)                  # [128, 4*HID]

    # csm layout: [mask | perm | ones | (cos_c | sin_c) per s-chunk]
    cosT = np.tile(cos[:S].T, (2, 1)).astype(NP_BF16)   # [128, S]
    sinT_h = np.concatenate([-sin[:S].T[:D // 2], sin[:S].T[D // 2:]], axis=0)
    sinT = np.tile(sinT_h, (2, 1)).astype(NP_BF16)
    trimask = np.triu(np.ones((P, P), dtype=NP_BF16))
    perm = np.zeros((P, P), dtype=NP_BF16)
    for j in range(P):
        base = (j // 64) * 64
        perm[base + ((j - base + 32) % 64), j] = 1
    onesb = np.ones((P, P), dtype=NP_BF16)
    cs_blocks = []
    for c in range(NSC):
        cs_blocks.append(cosT[:, c * SC:(c + 1) * SC])
        cs_blocks.append(sinT[:, c * SC:(c + 1) * SC])
    csm = np.ascontiguousarray(
        np.concatenate([trimask, perm, onesb] + cs_blocks, axis=1)
    )

    return {"xTp": xTp, "wqkvp": wqkvp, "csm": csm, "wop": wop}


def kernel(x, cos, sin, wq, wk, wv, wo):
    x = np.asarray(x)
    S = x.shape[1]
    assert x.shape == (B, S, HID)
    if S not in _CACHE:
        _CACHE[S] = build_nc(S)
    nc = _CACHE[S]
    in_maps = [
        prep_core_inputs(x, np.asarray(cos), np.asarray(sin), np.asarray(wq),
                         np.asarray(wk), np.asarray(wv), np.asarray(wo), core)
        for core in range(8)
    ]
    res = run_bass_kernel_spmd(nc, in_maps, core_ids=list(range(8)))
    out = np.zeros((B, S, HID), np.float32)
    for core in range(8):
        out[core // 4] += res.results[core]["o_part"].astype(np.float32)
    return out
